# revision 6
# baseline (speedup 1.0000x reference)
"""Trainium2 Bass kernel: top-2 MoE (8 experts, E=1024, H=1536, T=16384).

Sharding: expert-parallel with 2-segment load balancing, host-routed.
The router (0.07% of model FLOPs) runs on the host in fp32; the host
dispatches tokens by topk_idx. Global per-expert counts fluctuate around
4096 (max ~4340 for the reference input), so a plain one-expert-per-core
split pads every core to the max. Instead each core processes two
statically-sized segments, each with its own expert weights:

  segment A (SA tokens): the first SA tokens routed to expert c
  segment B (SB tokens): one overflow piece - leftover tokens of any
    expert whose count exceeds SA (assignment solved on host; B slots
    are interchangeable across cores)

(SA, SB) are the smallest feasible pair (Σ_e ceil((N_e-SA)+/SB) <= 8),
so per-core work is ~4224 tokens instead of max_e N_e ~= 4340.

Each segment is a fully dense FFN with the token count as the matmul
*free* dimension in both GEMMs (no 128-token padding, no on-device
gather/scatter, no gpsimd):

    H^T = gelu(W1^T X^T + b1)    [1536, n]  (12 h-tiles, 8 k-tiles)
    Y^T = W2^T H^T               [1024, n]  ( 8 f-tiles, 12 k-tiles)

streamed in <=512-token chunks (one PSUM bank per accumulation; FWL
keeps back-to-back 512-free matmuls at ~216ns measured). The first
chunk is 256 tokens so the first matmul starts ~5us earlier; the last
chunk's output store is split per f-tile to shorten the kernel-tail
drain. Y^T is written back compacted (bf16); the host applies the fp32
softmax gates and b2 while combining the two expert contributions per
token, so the device does 99.9% of the FLOPs (the GEMMs) and nothing
else.

The Bass program depends only on (SA, SB); it is rebuilt (recompiled)
if a different input's routing needs different segment sizes.
"""

import numpy as np
import ml_dtypes

import concourse.bacc as bacc
import concourse.mybir as mybir
import concourse.tile as tile
from concourse.bass_utils import run_bass_kernel_spmd

F32 = mybir.dt.float32
BF16 = mybir.dt.bfloat16
AF = mybir.ActivationFunctionType

B, N, E, H, NE = 8, 2048, 1024, 1536, 8
T = B * N
KT = E // 128          # 8 k-tiles of input features
HT = H // 128          # 12 tiles of hidden
FT = E // 128          # 8 output feature tiles
TOP_K = 2
CHUNK = 512

_CACHE = {}


def _chunk_sizes(n, first_small, last_small=False):
    """Split n into chunks <= 512, optionally with small first/last chunks."""
    sizes = []
    if first_small and n > 256:
        sizes.append(256)
        n -= 256
    if last_small and n > 128:
        tail = [128]
        n -= 128
    else:
        tail = []
    while n > 0:
        c = min(n, CHUNK)
        sizes.append(c)
        n -= c
    return sizes + tail


def _build_nc(sa, sb):
    nc = bacc.Bacc("TRN2", target_bir_lowering=False)
    cap = sa + sb
    xT = nc.dram_tensor("xT", [128, KT, cap], BF16, kind="ExternalInput")
    # W1 staged h-tile-major so the first h-tile's weights (256KB) land fast
    w1 = nc.dram_tensor("w1", [128, HT, KT, 128], BF16, kind="ExternalInput")
    w2 = nc.dram_tensor("w2", [128, HT, E], BF16, kind="ExternalInput")
    b1v = nc.dram_tensor("b1v", [128, HT], F32, kind="ExternalInput")
    if sb:
        w1b = nc.dram_tensor("w1b", [128, HT, KT, 128], BF16, kind="ExternalInput")
        w2b = nc.dram_tensor("w2b", [128, HT, E], BF16, kind="ExternalInput")
        b1vb = nc.dram_tensor("b1vb", [128, HT], F32, kind="ExternalInput")
    yt = nc.dram_tensor("yt", [128, FT, cap], BF16, kind="ExternalOutput")
    # "head": W1 h-tile 0 + the first 256-token x chunk packed contiguous
    # per partition, so the startup loads run at 2KB-descriptor bandwidth
    xh = nc.dram_tensor("xh", [128, KT * 128 + KT * 256], BF16,
                        kind="ExternalInput")

    # (chunk_size, c0, segment) schedule; segment 0 = A, 1 = B
    sched = []
    c0 = 0
    for cw in _chunk_sizes(sa, first_small=True, last_small=(sb == 0)):
        sched.append((cw, c0, 0))
        c0 += cw
    for cw in _chunk_sizes(sb, first_small=False, last_small=True):
        sched.append((cw, c0, 1))
        c0 += cw

    with tile.TileContext(nc) as tc:
        with (
            tc.tile_pool(name="consts", bufs=1) as cpool,
            tc.tile_pool(name="xc", bufs=3) as x_pool,
            tc.tile_pool(name="h", bufs=2) as h_pool,
            tc.tile_pool(name="y", bufs=2) as y_pool,
            tc.tile_pool(name="psH", bufs=2, space="PSUM") as psH_pool,
            tc.tile_pool(name="psY", bufs=2, space="PSUM") as psY_pool,
        ):
            # Startup data (first weight h-tile + first two x chunks) gates
            # the first matmuls, but DMA triggers serialize at ~650ns each on
            # a single engine. Fan the first triggers across the four idle
            # engines so they fire in parallel right after the prologue
            # barrier, and split the transfers over parallel DMA queues.
            w1_sb = cpool.tile([128, HT, KT, 128], BF16)
            xh_sb = cpool.tile([128, KT * 128 + KT * 256], BF16)
            third = KT * 128  # 1024; remaining 2048 split in two
            nc.sync.dma_start(xh_sb[:, 0:third], xh[:, 0:third])
            nc.scalar.dma_start(xh_sb[:, third:2 * third],
                                xh[:, third:2 * third])
            nc.gpsimd.dma_start(xh_sb[:, 2 * third:], xh[:, 2 * third:])

            xc_pre = [None]
            for ci, ksplit in ((1, 2),):
                cw, c0, _ = sched[ci]
                xc = x_pool.tile([128, KT, CHUNK], BF16, tag="xc")
                kk = KT // ksplit
                for g in range(ksplit):
                    nc.sync.dma_start(
                        xc[:, g * kk:(g + 1) * kk, 0:cw],
                        xT[:, g * kk:(g + 1) * kk, c0:c0 + cw])
                xc_pre.append(xc)

            b1_sb = cpool.tile([128, HT], F32)
            nc.gpsimd.dma_start(b1_sb[:], b1v[:])

            # h-tiles 1-2 off the serial sync queue: chunk 0 consumes one
            # h-tile per ~1.5us and sync alone streams them too late
            # (measured 2.7us stall on the h-tile-1 DMA semaphore)
            nc.scalar.dma_start(w1_sb[:, 1], w1[:, 1])
            nc.gpsimd.dma_start(w1_sb[:, 2], w1[:, 2])
            nc.sync.dma_start(w1_sb[:, 0], w1[:, 0])  # chunks >=1 reuse h0
    # noqa
            for hb in range(3, HT):
                nc.sync.dma_start(w1_sb[:, hb], w1[:, hb])
            w2_sb = cpool.tile([128, HT, E], BF16)
            for k2 in range(HT):
                nc.sync.dma_start(w2_sb[:, k2], w2[:, k2])
            if sb:
                b1b_sb = cpool.tile([128, HT], F32)
                nc.sync.dma_start(b1b_sb[:], b1vb[:])
                w1b_sb = cpool.tile([128, HT, KT, 128], BF16)
                nc.sync.dma_start(w1b_sb[:], w1b[:])
                w2b_sb = cpool.tile([128, HT, E], BF16)
                nc.sync.dma_start(w2b_sb[:], w2b[:])

            for ci, (cw, c0, seg) in enumerate(sched):
                wa, wb, bb = (w1_sb, w2_sb, b1_sb) if seg == 0 else \
                             (w1b_sb, w2b_sb, b1b_sb)
                use_head = ci == 0 and cw == 256
                if ci < 2 and not (ci == 0 and not use_head):
                    xc = xc_pre[ci]
                else:
                    xc = x_pool.tile([128, KT, CHUNK], BF16, tag="xc")
                    nc.sync.dma_start(xc[:, :, 0:cw], xT[:, :, c0:c0 + cw])
                hT = h_pool.tile([128, HT, CHUNK], BF16, tag="hT")
                for hb in range(HT):
                    ps = psH_pool.tile([128, cw], F32, tag="psH")
                    for k in range(KT):
                        if use_head:
                            lw = xh_sb[:, 128 * k:128 * (k + 1)] if hb == 0                                 else wa[:, hb, k, :]
                            rh = xh_sb[:, KT * 128 + 256 * k:
                                       KT * 128 + 256 * (k + 1)]
                        else:
                            lw, rh = wa[:, hb, k, :], xc[:, k, 0:cw]
                        nc.tensor.matmul(
                            ps[:], lhsT=lw, rhs=rh,
                            start=(k == 0), stop=(k == KT - 1))
                    nc.scalar.activation(hT[:, hb, 0:cw], ps[:], AF.Gelu,
                                         bias=bb[:, hb:hb + 1])
                yc = y_pool.tile([128, FT, CHUNK], BF16, tag="yc")
                for f in range(FT):
                    ps = psY_pool.tile([128, cw], F32, tag="psY")
                    for k2 in range(HT):
                        nc.tensor.matmul(
                            ps[:], lhsT=wb[:, k2, 128 * f:128 * (f + 1)],
                            rhs=hT[:, k2, 0:cw],
                            start=(k2 == 0), stop=(k2 == HT - 1))
                    nc.vector.tensor_copy(yc[:, f, 0:cw], ps[:])
                nc.sync.dma_start(yt[:, :, c0:c0 + cw], yc[:, :, 0:cw])
    return nc


def get_nc(sa, sb):
    if _CACHE.get("key") != (sa, sb):
        nc = _build_nc(sa, sb)
        nc.finalize()
        _CACHE["key"] = (sa, sb)
        _CACHE["nc"] = nc
    return _CACHE["nc"]


def _route(x, Wr, br):
    """Host router: fp32 logits, top-2, fp64 softmax gates."""
    xf = np.ascontiguousarray(x.reshape(T, E), dtype=np.float32)
    logits = xf @ Wr.astype(np.float32) + br.astype(np.float32)      # [T, NE]
    top2 = np.argsort(-logits, axis=1, kind="stable")[:, :TOP_K]     # [T, 2]
    z = (logits - logits.max(axis=1, keepdims=True)).astype(np.float64)
    p = np.exp(z)
    p /= p.sum(axis=1, keepdims=True)
    gates = np.take_along_axis(p, top2, axis=1).astype(np.float32)   # [T, 2]
    return top2, gates


def _pick_segments(counts):
    """Smallest (SA, SB) with one A piece per expert and <= NE overflow
    B pieces of size SB; (cap16(max), 0) is the single-segment fallback."""
    best = (int(-(-counts.max() // 16)) * 16, 0)
    for sb in (128, 256, 384, 512):
        for sa in range(3584, int(counts.max()) + 64, 64):
            if sa + sb >= best[0] + best[1]:
                break
            pieces = int(sum(-(-max(int(c) - sa, 0) // sb) for c in counts))
            if pieces <= NE:
                best = (sa, sb)
                break
    return best


def run(inputs, **kw):
    x = np.asarray(inputs["x"], dtype=np.float32)
    Wr = np.asarray(inputs["Wr"], dtype=np.float32)
    br = np.asarray(inputs["br"], dtype=np.float32)
    W1 = np.asarray(inputs["W1"], dtype=np.float32)
    b1 = np.asarray(inputs["b1"], dtype=np.float32)
    W2 = np.asarray(inputs["W2"], dtype=np.float32)
    b2 = np.asarray(inputs["b2"], dtype=np.float32)
    assert x.shape == (B, N, E) and W1.shape == (NE, E, H) and W2.shape == (NE, H, E)

    top2, gates = _route(x, Wr, br)

    bf = ml_dtypes.bfloat16
    xb = x.reshape(T, E).astype(bf)

    toks, posmap = [], np.empty((NE, T), dtype=np.int64)
    for e in range(NE):
        tok_e = np.nonzero((top2 == e).any(axis=1))[0]
        toks.append(tok_e)
        posmap[e, tok_e] = np.arange(len(tok_e))
    counts = np.array([len(t) for t in toks])
    sa, sb = _pick_segments(counts)
    cap = sa + sb

    # core c runs expert c's first <=SA tokens as segment A; overflow
    # pieces fill the B slots (first-fit over cores).
    seg_b = [None] * NE            # per core: (expert, start_in_tok_e, len)
    core_of = np.zeros((NE, T), dtype=np.int64)   # (expert, idx_in_e) -> core
    pos_of = np.zeros((NE, T), dtype=np.int64)    # (expert, idx_in_e) -> pos
    free_b = list(range(NE))
    for e in range(NE):
        na = min(counts[e], sa)
        core_of[e, :na] = e
        pos_of[e, :na] = np.arange(na)
        off = na
        while off < counts[e]:
            ln = min(counts[e] - off, sb)
            c = free_b.pop(0)
            seg_b[c] = (e, off, ln)
            core_of[e, off:off + ln] = c
            pos_of[e, off:off + ln] = sa + np.arange(ln)
            off += ln

    def stage_w(W1e, W2e, b1e, sfx):
        return {
            "w1" + sfx: np.ascontiguousarray(
                W1e.reshape(KT, 128, HT, 128).transpose(1, 2, 0, 3).astype(bf)),
            "w2" + sfx: np.ascontiguousarray(
                W2e.reshape(HT, 128, E).transpose(1, 0, 2).astype(bf)),
            "b1v" + sfx: np.ascontiguousarray(b1e.reshape(HT, 128).T),
        }

    in_maps = []
    for c in range(NE):
        Xg = np.zeros((cap, E), dtype=bf)
        na = min(counts[c], sa)
        Xg[:na] = xb[toks[c][:na]]
        m = {"xT": None}
        m.update(stage_w(W1[c], W2[c], b1[c], ""))
        if sb:
            if seg_b[c] is not None:
                e, off, ln = seg_b[c]
                Xg[sa:sa + ln] = xb[toks[e][off:off + ln]]
                m.update(stage_w(W1[e], W2[e], b1[e], "b"))
            else:
                m.update(stage_w(W1[c], W2[c], b1[c], "b"))
        m["xT"] = np.ascontiguousarray(Xg.reshape(cap, KT, 128).transpose(2, 1, 0))
        m["xh"] = np.ascontiguousarray(np.concatenate(
            [m["w1"][:, 0].reshape(128, -1),
             m["xT"][:, :, 0:256].reshape(128, -1)], axis=1))
        in_maps.append(m)

    nc = get_nc(sa, sb)
    res = run_bass_kernel_spmd(nc, in_maps, list(range(NE)), **kw)

    # host combine: out[t] = sum_s gates[t,s] * (Y[core, pos] + b2[expert])
    Yall = np.empty((NE, cap, E), dtype=np.float32)
    for c in range(NE):
        yt_c = np.asarray(res.results[c]["yt"], dtype=np.float32)    # [128, FT, cap]
        Yall[c] = yt_c.transpose(2, 1, 0).reshape(cap, E)
    out = np.zeros((T, E), dtype=np.float32)
    tr = np.arange(T)
    for s in range(TOP_K):
        es = top2[:, s]
        ie = posmap[es, tr]
        out += gates[:, s:s + 1] * (Yall[core_of[es, ie], pos_of[es, ie]] + b2[es])
    return out.reshape(B, N, E), res


def kernel(**inputs):
    out, _ = run(inputs)
    return out
